# revision 1
# baseline (speedup 1.0000x reference)
"""CPC InfoNCE loss kernel for Trainium2 (8 NeuronCores, data-parallel rows).

Per core (rows sharded across cores, 3 horizons x 8 blocks of 128 rows):
  - Host normalizes the pool table all_z = normalize(z_seq.reshape(BT, D)) and
    uploads it transposed in bf16 (AZT). Host also gathers per-core anchor and
    positive rows (transposed, bf16), the predictor weights (transposed, bf16),
    and a dense per-row count matrix C [row, pool] (bf16; multiplicity of each
    pool entry among the row's 128 sampled negatives, plus 1 at the positive).
  - PE computes U^T = W @ Z_anchor^T, per-row norms ||u||^2 via a ones-matmul,
    and the positive logits via a ones-matmul over ut*az_pos products.
  - For each 128-row block PE computes the full similarity block
    S = U_blk @ AZT into PSUM; ACT applies exp(scale*S) straight out of PSUM
    (scale = 1/(tau*||u||) per row) into a bf16 SBUF tile.
  - DVE multiplies by the C tile (zeroing the ~98.4% unsampled entries,
    weighting duplicates) and reduces each row to R = sum_j e^{s_j}
    (positive included via its count). loss = ln(R) - s_pos per row.
  - Host averages the returned [128, 24] per-row losses with the horizon
    weights (the unshard step).
"""

import sys

sys.path.insert(0, "/opt/trn_rl_repo")

import math
import os

import ml_dtypes
import numpy as np

import concourse.bass as bass
import concourse.tile as tile
from concourse import bacc
from concourse import mybir
from concourse.bass_utils import run_bass_kernel_spmd

# Problem constants (hardcoded per contract)
B, T, D = 16, 512, 256
BT = B * T  # 8192 pool entries
HORIZONS = (1, 5, 21)
H = len(HORIZONS)
N_NEG = 128
TAU = 0.07
N_CORES = 8

P = 128
NROW = 1024  # padded rows per core per horizon
NBLK = NROW // P  # 8
NCOL = H * NBLK  # 24 row-blocks per core
POOL_TILE = 512
N_PTILES = BT // POOL_TILE  # 16

BF16 = mybir.dt.bfloat16
F32 = mybir.dt.float32


def _split_multiwait_drains(nc):
    """This walrus build accepts only one sync-wait command per TPB_CTRL
    instruction; TileContext's exit drain carries one wait per live proc.
    Split the extras into preceding single-wait drains."""
    for f in nc.m.functions:
        for bb in f.blocks:
            new_list = []
            for inst in bb.instructions:
                si = inst.sync_info
                if si is not None and si.on_wait and len(si.on_wait) > 1:
                    waits = list(si.on_wait)
                    for j, w in enumerate(waits[:-1]):
                        d = mybir.InstDrain(
                            name=f"{inst.name}-w{j}", ins=[], outs=[]
                        )
                        d.engine = inst.engine
                        d.sync_info = mybir.SyncInfo(on_wait=[w], on_update=[])
                        nc.register_instruction(d)
                        new_list.append(d)
                    si.on_wait = [waits[-1]]
                    inst.sync_info = si
                new_list.append(inst)
            bb.instructions[:] = new_list


def build_program(reps=1):
    reps = int(os.environ.get("KERNEL_REPS", reps))
    nc = bacc.Bacc(
        "TRN2", target_bir_lowering=False, debug=False, num_devices=N_CORES
    )

    azt_d = nc.declare_dram_parameter("azt", [P, 2, BT], BF16, isOutput=False)
    zat_d = nc.declare_dram_parameter("zat", [P, H * 2, NROW], BF16, isOutput=False)
    azp_d = nc.declare_dram_parameter("azp", [P, H * 2, NROW], BF16, isOutput=False)
    pt_d = nc.declare_dram_parameter("pt", [P, H * 4, P], BF16, isOutput=False)
    cnt_d = nc.declare_dram_parameter("cnt", [P, NCOL, BT], BF16, isOutput=False)
    loss_d = nc.declare_dram_parameter("loss", [P, NCOL], F32, isOutput=True)

    from contextlib import ExitStack, nullcontext

    with tile.TileContext(nc) as tc, ExitStack() as ctx:
        singles = ctx.enter_context(tc.tile_pool(name="singles", bufs=1))
        ut_pool = ctx.enter_context(tc.tile_pool(name="ut", bufs=2))
        c_pool = ctx.enter_context(tc.tile_pool(name="c", bufs=2))
        e_pool = ctx.enter_context(tc.tile_pool(name="e", bufs=2))
        small = ctx.enter_context(tc.tile_pool(name="small", bufs=2))
        junk_pool = ctx.enter_context(tc.tile_pool(name="junk", bufs=1))
        psum_s = ctx.enter_context(tc.tile_pool(name="psum_s", bufs=2, space="PSUM"))
        psum_u = ctx.enter_context(tc.tile_pool(name="psum_u", bufs=1, space="PSUM"))
        psum_b = ctx.enter_context(tc.tile_pool(name="psum_b", bufs=1, space="PSUM"))
        psum_r = ctx.enter_context(tc.tile_pool(name="psum_r", bufs=1, space="PSUM"))

        # ---- preload constants -------------------------------------------
        azt_sb = singles.tile([P, 2, BT], BF16)
        nc.sync.dma_start(out=azt_sb[:], in_=azt_d[:])
        zat_sb = singles.tile([P, H * 2, NROW], BF16)
        nc.sync.dma_start(out=zat_sb[:], in_=zat_d[:])
        azp_sb = singles.tile([P, H * 2, NROW], BF16)
        nc.sync.dma_start(out=azp_sb[:], in_=azp_d[:])
        pt_sb = singles.tile([P, H * 4, P], BF16)
        nc.sync.dma_start(out=pt_sb[:], in_=pt_d[:])

        ones_sb = singles.tile([P, 1], BF16)
        nc.vector.memset(ones_sb[:], 1.0)
        one1_sb = singles.tile([1, 1], F32)
        nc.vector.memset(one1_sb[:], 1.0)

        loss_sb = singles.tile([P, NCOL], F32)
        rsum_sb = singles.tile([P, NCOL], F32)
        rsT_sb = singles.tile([P, NCOL], F32)
        spT_sb = singles.tile([P, NCOL], F32)

        loop_cm = tc.For_i(0, reps, 1) if reps > 1 else nullcontext()
        with loop_cm:
            for i in range(H):
                # ---- predictions U^T + per-row norm / positive logit -----
                ut_sb = ut_pool.tile([P, 2, NROW], BF16, tag="ut")
                rs_flat = small.tile([1, NROW], F32, tag="rsflat")
                sp_flat = small.tile([1, NROW], F32, tag="spflat")
                nsum = small.tile([1, NROW], F32, tag="nsum")
                for mc in range(2):
                    for nh in range(2):  # one PSUM bank per matmul
                        nsl = slice(nh * (NROW // 2), (nh + 1) * (NROW // 2))
                        pu = psum_u.tile([P, NROW // 2], F32, tag="pu")
                        for kc in range(2):
                            nc.tensor.matmul(
                                pu[:],
                                pt_sb[:, i * 4 + kc * 2 + mc, :],
                                zat_sb[:, i * 2 + kc, nsl],
                                start=(kc == 0),
                                stop=(kc == 1),
                            )
                        # bf16 copy for the S-matmul lhsT
                        nc.scalar.copy(out=ut_sb[:, mc, nsl], in_=pu[:])
                    # squared entries (from the bf16-rounded values used below)
                    usq = junk_pool.tile([P, NROW], BF16, tag="usq")
                    nc.vector.tensor_mul(usq[:], ut_sb[:, mc, :], ut_sb[:, mc, :])
                    # ut * az_pos products for the positive logits
                    upr = junk_pool.tile([P, NROW], BF16, tag="upr")
                    nc.vector.tensor_mul(
                        upr[:], ut_sb[:, mc, :], azp_sb[:, i * 2 + mc, :]
                    )
                    # column sums via ones-matmuls, accumulated in SBUF
                    for nh in range(2):
                        nsl = slice(nh * (NROW // 2), (nh + 1) * (NROW // 2))
                        pb_n = psum_b.tile([1, NROW // 2], F32, tag="pbn")
                        pb_p = psum_b.tile([1, NROW // 2], F32, tag="pbp")
                        nc.tensor.matmul(
                            pb_n[:], ones_sb[:], usq[:, nsl],
                            start=True, stop=True,
                        )
                        nc.tensor.matmul(
                            pb_p[:], ones_sb[:], upr[:, nsl],
                            start=True, stop=True,
                        )
                        if mc == 0:
                            nc.vector.tensor_copy(out=nsum[0:1, nsl], in_=pb_n[:])
                            nc.vector.tensor_copy(out=sp_flat[0:1, nsl], in_=pb_p[:])
                        else:
                            nc.vector.tensor_add(
                                out=nsum[0:1, nsl], in0=nsum[0:1, nsl], in1=pb_n[:]
                            )
                            nc.vector.tensor_add(
                                out=sp_flat[0:1, nsl], in0=sp_flat[0:1, nsl],
                                in1=pb_p[:],
                            )
                # rs_flat = 1/(tau*||u||) = 1/sqrt(tau^2 * ||u||^2)
                nc.scalar.activation(
                    out=rs_flat[:], in_=nsum[:],
                    func=mybir.ActivationFunctionType.Sqrt,
                    scale=float(TAU * TAU),
                )
                nc.vector.reciprocal(out=rs_flat[:], in_=rs_flat[:])
                # sp_flat = raw_pos_dot * rs  (the positive logit)
                nc.vector.tensor_mul(sp_flat[:], sp_flat[:], rs_flat[:])
                # transpose the per-row scalars into per-block columns
                for rb in range(NBLK):
                    col = i * NBLK + rb
                    pr = psum_r.tile([P, 2], F32, tag="pr")
                    nc.tensor.matmul(
                        pr[:, 0:1], rs_flat[0:1, rb * P:(rb + 1) * P],
                        one1_sb[:], start=True, stop=True,
                    )
                    nc.tensor.matmul(
                        pr[:, 1:2], sp_flat[0:1, rb * P:(rb + 1) * P],
                        one1_sb[:], start=True, stop=True,
                    )
                    nc.scalar.copy(out=rsT_sb[:, col:col + 1], in_=pr[:, 0:1])
                    nc.scalar.copy(out=spT_sb[:, col:col + 1], in_=pr[:, 1:2])

                # ---- per row-block: S matmul -> exp -> masked reduce -----
                for rb in range(NBLK):
                    col = i * NBLK + rb
                    c_sb = c_pool.tile([P, BT], BF16, tag="c")
                    nc.sync.dma_start(out=c_sb[:], in_=cnt_d[:, col, :])
                    e_sb = e_pool.tile([P, BT], BF16, tag="e")
                    for ph in range(N_PTILES // 2):
                        ps = psum_s.tile([P, 2 * POOL_TILE], F32, tag="ps")
                        for sub in range(2):
                            pt_i = ph * 2 + sub
                            for kc in range(2):
                                nc.tensor.matmul(
                                    ps[:, sub * POOL_TILE:(sub + 1) * POOL_TILE],
                                    ut_sb[:, kc, rb * P:(rb + 1) * P],
                                    azt_sb[:, kc,
                                           pt_i * POOL_TILE:(pt_i + 1) * POOL_TILE],
                                    start=(kc == 0),
                                    stop=(kc == 1),
                                )
                        # exp straight out of PSUM (fused copy+scale+exp)
                        nc.scalar.activation(
                            out=e_sb[:, ph * 2 * POOL_TILE:(ph + 1) * 2 * POOL_TILE],
                            in_=ps[:],
                            func=mybir.ActivationFunctionType.Exp,
                            scale=rsT_sb[:, col:col + 1],
                        )
                    # R = sum_m cnt[m] * e[m]  (counts include the positive),
                    # fused multiply + free-dim accumulate on DVE
                    nc.vector.scalar_tensor_tensor(
                        out=e_sb[:], in0=e_sb[:], scalar=1.0, in1=c_sb[:],
                        op0=mybir.AluOpType.mult, op1=mybir.AluOpType.mult,
                        accum_out=rsum_sb[:, col:col + 1],
                    )
            # loss = ln(R) - s_pos, batched over all 24 columns
            nc.scalar.activation(
                out=loss_sb[:], in_=rsum_sb[:],
                func=mybir.ActivationFunctionType.Ln,
            )
            nc.vector.tensor_tensor(
                loss_sb[:], loss_sb[:], spT_sb[:], mybir.AluOpType.subtract,
            )

        nc.sync.dma_start(out=loss_d[:], in_=loss_sb[:])

    nc.compile()
    _split_multiwait_drains(nc)
    return nc


def prepare_inputs(z_seq, preds, neg_idx):
    """Host-side sharding/packing. Returns (in_maps, valid_counts)."""
    z_flat = np.asarray(z_seq, dtype=np.float32).reshape(BT, D)
    preds = np.asarray(preds, dtype=np.float32)
    neg_idx = np.asarray(neg_idx)

    norms = np.linalg.norm(z_flat, axis=1, keepdims=True)
    az = z_flat / np.maximum(norms, 1e-12)
    azt = np.ascontiguousarray(
        az.T.reshape(2, P, BT).transpose(1, 0, 2)
    ).astype(ml_dtypes.bfloat16)

    # pt[d, i*4+kc*2+mc, e] = preds[i, mc*128+e, kc*128+d]
    pt = np.empty((P, H * 4, P), dtype=ml_dtypes.bfloat16)
    for i in range(H):
        w = preds[i]  # [e_out, d_in]
        for kc in range(2):
            for mc in range(2):
                blk = w[mc * P:(mc + 1) * P, kc * P:(kc + 1) * P]  # [e, d]
                pt[:, i * 4 + kc * 2 + mc, :] = blk.T.astype(ml_dtypes.bfloat16)

    in_maps = []
    valid_counts = np.zeros((N_CORES, H), dtype=np.int64)
    for c in range(N_CORES):
        n0 = c * NROW
        zat = np.zeros((P, H * 2, NROW), dtype=ml_dtypes.bfloat16)
        azp = np.zeros((P, H * 2, NROW), dtype=ml_dtypes.bfloat16)
        cnt = np.zeros((P, NCOL, BT), dtype=ml_dtypes.bfloat16)
        for i, k in enumerate(HORIZONS):
            L = T - k
            BL = B * L
            nvalid = min(max(BL - n0, 0), NROW)
            valid_counts[c, i] = nvalid
            n = n0 + np.arange(NROW)
            nv = n[:nvalid]
            b = nv // L
            a_full = np.zeros(NROW, dtype=np.int64)
            a_full[:nvalid] = nv + b * k          # anchor flat rows
            p_full = np.zeros(NROW, dtype=np.int64)
            p_full[:nvalid] = nv + (b + 1) * k    # positive flat rows
            zat[:, i * 2:(i + 1) * 2, :] = (
                z_flat[a_full].T.reshape(2, P, NROW).transpose(1, 0, 2)
            ).astype(ml_dtypes.bfloat16)
            azp[:, i * 2:(i + 1) * 2, :] = (
                az[p_full].T.reshape(2, P, NROW).transpose(1, 0, 2)
            ).astype(ml_dtypes.bfloat16)

            # dense counts: negatives multiplicity + 1 at the positive
            cm = np.zeros((NROW, BT), dtype=np.float32)
            rows = np.repeat(np.arange(nvalid), N_NEG)
            np.add.at(cm, (rows, neg_idx[i, nv, :].reshape(-1)), 1.0)
            cm[np.arange(NROW), p_full] += 1.0
            if nvalid < NROW:
                # pad rows: keep a single count so R>0 (host ignores them)
                cm[nvalid:] = 0.0
                cm[nvalid:, 0] = 1.0
            cmb = cm.astype(ml_dtypes.bfloat16)
            for rb in range(NBLK):
                cnt[:, i * NBLK + rb, :] = cmb[rb * P:(rb + 1) * P]

        in_maps.append({"azt": azt, "zat": zat, "azp": azp, "pt": pt, "cnt": cnt})
    return in_maps, valid_counts


def reduce_outputs(results, valid_counts):
    raw_w = {k: 1.0 / math.sqrt(k) for k in HORIZONS}
    tot_w = sum(raw_w.values())
    total = np.float64(0.0)
    for i, k in enumerate(HORIZONS):
        L = T - k
        BL = B * L
        s = np.float64(0.0)
        for c in range(N_CORES):
            nvalid = int(valid_counts[c, i])
            if nvalid == 0:
                continue
            lm = results[c]["loss"]  # [P, NCOL]
            per_row = lm[:, i * NBLK:(i + 1) * NBLK].T.reshape(NROW)
            s += per_row[:nvalid].sum(dtype=np.float64)
        total += (raw_w[k] / tot_w) * (s / BL)
    return np.float32(total)


_CACHED_NC = None


def kernel(z_seq, preds, neg_idx):
    global _CACHED_NC
    if _CACHED_NC is None:
        _CACHED_NC = build_program()
    nc = _CACHED_NC
    in_maps, valid_counts = prepare_inputs(z_seq, preds, neg_idx)
    res = run_bass_kernel_spmd(nc, in_maps, list(range(N_CORES)))
    return reduce_outputs(res.results, valid_counts)


if __name__ == "__main__":
    rng = np.random.default_rng(0)
    z = rng.standard_normal((B, T, D), dtype=np.float32)
    pr = (rng.standard_normal((H, D, D), dtype=np.float32) / np.sqrt(D)).astype(
        np.float32
    )
    ni = rng.integers(0, BT, size=(H, BT, N_NEG), dtype=np.int64)
    print(kernel(z, pr, ni))



# revision 6
# speedup vs baseline: 1.0566x; 1.0566x over previous
"""CPC InfoNCE loss kernel for Trainium2 (8 NeuronCores, data-parallel rows).

Per core (rows sharded across cores, 3 horizons x 8 blocks of 128 rows):
  - Host normalizes the pool table all_z = normalize(z_seq.reshape(BT, D)),
    scales it by 8 and uploads it transposed in fp8-e4m3 (AZT, DoubleRow kc
    planes). Host also gathers per-core anchor and positive rows (transposed,
    bf16), the predictor weights (transposed, bf16), and a dense log-count
    matrix M [row, pool] in fp8 (ln of the multiplicity of each pool entry
    among the row's sampled negatives + positive; -104 where unsampled, so
    exp maps it to ~0).
  - PE computes U^T = W @ Z_anchor^T (bf16); per-row norms ||u||^2 and raw
    positive dots via ones-matmuls; rs8 = 1/(8*tau*||u||) per row.
  - DVE scales U by rs8 (broadcast via a PE ones-outer-product) into fp8.
    The scaled-U fp8 DoubleRow matmul against AZT then produces the final
    logits directly in PSUM, and a second fp8 DoubleRow matmul with an
    identity lhsT accumulates M on top: PSUM = logit + ln(count).
  - ACT applies a plain Exp out of PSUM (bf16 out); DVE reduces each row
    with a tensor_scalar accumulate: R = sum_m count*e^logit per row-block.
  - loss = ln(R) - pos_logit per row; host averages with horizon weights.
"""

import sys

sys.path.insert(0, "/opt/trn_rl_repo")

import math
import os

import ml_dtypes
import numpy as np

import concourse.bass as bass
import concourse.tile as tile
from concourse import bacc
from concourse import mybir
from concourse.bass_utils import run_bass_kernel_spmd

# Problem constants (hardcoded per contract)
B, T, D = 16, 512, 256
BT = B * T  # 8192 pool entries
HORIZONS = (1, 5, 21)
H = len(HORIZONS)
N_NEG = 128
TAU = 0.07
N_CORES = 8

P = 128
NROW = 1024  # padded rows per core per horizon
NBLK = NROW // P  # 8
NCOL = H * NBLK  # 24 row-blocks per core
NPAIR = NCOL // 2  # 12 block-pairs (DoubleRow M planes)
PTILE = 1024  # pool columns per PSUM tile (2 banks)
NPT = BT // PTILE  # 8
NEG_M = -104.0  # exp(logit + NEG_M) == 0 for unsampled entries
FP8_SCALE = 8.0  # az stored as az*8; u scaled by rs/8

BF16 = mybir.dt.bfloat16
F32 = mybir.dt.float32
FP8 = mybir.dt.float8e4
E4M3 = ml_dtypes.float8_e4m3
MULT = mybir.AluOpType.mult
ADD = mybir.AluOpType.add
DR = mybir.MatmulPerfMode.DoubleRow


def _split_multiwait_drains(nc):
    """This walrus build accepts only one sync-wait command per TPB_CTRL
    instruction; TileContext's exit drain carries one wait per live proc.
    Split the extras into preceding single-wait drains."""
    for f in nc.m.functions:
        for bb in f.blocks:
            new_list = []
            for inst in bb.instructions:
                si = inst.sync_info
                if si is not None and si.on_wait and len(si.on_wait) > 1:
                    waits = list(si.on_wait)
                    for j, w in enumerate(waits[:-1]):
                        d = mybir.InstDrain(
                            name=f"{inst.name}-w{j}", ins=[], outs=[]
                        )
                        d.engine = inst.engine
                        d.sync_info = mybir.SyncInfo(on_wait=[w], on_update=[])
                        nc.register_instruction(d)
                        new_list.append(d)
                    si.on_wait = [waits[-1]]
                    inst.sync_info = si
                new_list.append(inst)
            bb.instructions[:] = new_list


def build_program(reps=1):
    reps = int(os.environ.get("KERNEL_REPS", reps))
    nc = bacc.Bacc(
        "TRN2", target_bir_lowering=False, debug=False, num_devices=N_CORES
    )

    azt_d = nc.declare_dram_parameter("azt", [P, 2, BT], FP8, isOutput=False)
    zat_d = nc.declare_dram_parameter("zat", [P, H * 2, NROW], BF16, isOutput=False)
    azp_d = nc.declare_dram_parameter("azp", [P, H * 2, NROW], BF16, isOutput=False)
    pt_d = nc.declare_dram_parameter("pt", [P, H * 4, P], BF16, isOutput=False)
    id8_d = nc.declare_dram_parameter("id8", [P, 2, 2, P], FP8, isOutput=False)
    mln_d = nc.declare_dram_parameter("mln", [P, NPAIR, 2, BT], FP8, isOutput=False)
    loss_d = nc.declare_dram_parameter("loss", [P, NCOL], F32, isOutput=True)

    from contextlib import ExitStack, nullcontext

    with tile.TileContext(nc) as tc, ExitStack() as ctx:
        singles = ctx.enter_context(tc.tile_pool(name="singles", bufs=1))
        work = ctx.enter_context(tc.tile_pool(name="work", bufs=2))
        m_pool = ctx.enter_context(tc.tile_pool(name="m", bufs=2))
        flats = ctx.enter_context(tc.tile_pool(name="flats", bufs=2))
        junk = ctx.enter_context(tc.tile_pool(name="junk", bufs=2))
        psum_s = ctx.enter_context(tc.tile_pool(name="psum_s", bufs=3, space="PSUM"))
        psum_u = ctx.enter_context(tc.tile_pool(name="psum_u", bufs=1, space="PSUM"))
        psum_sm = ctx.enter_context(tc.tile_pool(name="psum_sm", bufs=1, space="PSUM"))

        # ---- preload constants -------------------------------------------
        azt_sb = singles.tile([P, 2, BT], FP8)
        nc.sync.dma_start(out=azt_sb[:], in_=azt_d[:])
        zat_sb = singles.tile([P, H * 2, NROW], BF16)
        nc.sync.dma_start(out=zat_sb[:], in_=zat_d[:])
        azp_sb = singles.tile([P, H * 2, NROW], BF16)
        nc.sync.dma_start(out=azp_sb[:], in_=azp_d[:])
        pt_sb = singles.tile([P, H * 4, P], BF16)
        nc.sync.dma_start(out=pt_sb[:], in_=pt_d[:])
        id8_sb = singles.tile([P, 2, 2, P], FP8)
        nc.sync.dma_start(out=id8_sb[:], in_=id8_d[:])

        ones_sb = singles.tile([P, 1], BF16)
        nc.vector.memset(ones_sb[:], 1.0)
        onerow_sb = singles.tile([1, P], F32)
        nc.vector.memset(onerow_sb[:], 1.0)
        one1_sb = singles.tile([1, 1], F32)
        nc.vector.memset(one1_sb[:], 1.0)

        loss_sb = singles.tile([P, NCOL], F32)
        rsum_sb = singles.tile([P, NCOL], F32)
        spT_sb = singles.tile([P, NCOL], F32)
        racc_sb = singles.tile([P, NCOL, NPT], F32)

        loop_cm = tc.For_i(0, reps, 1) if reps > 1 else nullcontext()
        with loop_cm:
            for i in range(H):
                ut = work.tile([P, 2, NROW], BF16, tag="ut")
                uts = work.tile([P, 2, NROW], FP8, tag="uts")
                rsb = work.tile([P, NROW], BF16, tag="rsb")
                rs8 = flats.tile([1, NROW], F32, tag="rs")
                spf = flats.tile([1, NROW], F32, tag="sp")
                nsum = flats.tile([1, NROW], F32, tag="ns")

                # ---- U^T = W @ Z_anchor^T (bf16), copied out by DVE ------
                for mc in range(2):
                    for nh in range(2):
                        nsl = slice(nh * (NROW // 2), (nh + 1) * (NROW // 2))
                        pu = psum_u.tile([P, NROW // 2], F32, tag="pu")
                        for kc in range(2):
                            nc.tensor.matmul(
                                pu[:],
                                pt_sb[:, i * 4 + kc * 2 + mc, :],
                                zat_sb[:, i * 2 + kc, nsl],
                                start=(kc == 0),
                                stop=(kc == 1),
                            )
                        nc.vector.tensor_copy(out=ut[:, mc, nsl], in_=pu[:])

                # ---- per-row ||u||^2 and raw positive dot ----------------
                usq = []
                upr = []
                for mc in range(2):
                    sq = junk.tile([P, NROW], BF16, tag=f"usq{mc}")
                    nc.vector.tensor_mul(sq[:], ut[:, mc, :], ut[:, mc, :])
                    usq.append(sq)
                    pr_ = junk.tile([P, NROW], BF16, tag=f"upr{mc}")
                    nc.vector.tensor_mul(
                        pr_[:], ut[:, mc, :], azp_sb[:, i * 2 + mc, :]
                    )
                    upr.append(pr_)
                for nh in range(2):
                    nsl = slice(nh * (NROW // 2), (nh + 1) * (NROW // 2))
                    pbn = psum_sm.tile([1, NROW // 2], F32, tag="sm")
                    for mc in range(2):
                        nc.tensor.matmul(
                            pbn[:], ones_sb[:], usq[mc][:, nsl],
                            start=(mc == 0), stop=(mc == 1),
                        )
                    nc.vector.tensor_copy(out=nsum[0:1, nsl], in_=pbn[:])
                    pbp = psum_sm.tile([1, NROW // 2], F32, tag="sm")
                    for mc in range(2):
                        nc.tensor.matmul(
                            pbp[:], ones_sb[:], upr[mc][:, nsl],
                            start=(mc == 0), stop=(mc == 1),
                        )
                    nc.vector.tensor_copy(out=spf[0:1, nsl], in_=pbp[:])

                # rs8 = 1/(8*tau*||u||) = 1/sqrt(64*tau^2*||u||^2)
                nc.scalar.activation(
                    out=rs8[:], in_=nsum[:],
                    func=mybir.ActivationFunctionType.Sqrt,
                    scale=float(64.0 * TAU * TAU),
                )
                nc.vector.reciprocal_approx_fast(out=rs8[:], in_=rs8[:])
                # positive logit = raw_dot * rs8 * 8
                nc.vector.scalar_tensor_tensor(
                    out=spf[:], in0=spf[:], scalar=8.0, in1=rs8[:],
                    op0=MULT, op1=MULT,
                )
                # transpose positive logits into per-block columns
                for rb in range(NBLK):
                    col = i * NBLK + rb
                    pr = psum_sm.tile([P, 2], F32, tag="sm")
                    nc.tensor.matmul(
                        pr[:, 0:1], spf[0:1, rb * P:(rb + 1) * P],
                        one1_sb[:], start=True, stop=True,
                    )
                    nc.vector.tensor_copy(out=spT_sb[:, col:col + 1], in_=pr[:, 0:1])

                # broadcast rs8 across partitions -> bf16 [P, NROW]
                for nh in range(2):
                    nsl = slice(nh * (NROW // 2), (nh + 1) * (NROW // 2))
                    rr = psum_sm.tile([P, NROW // 2], F32, tag="sm")
                    nc.tensor.matmul(
                        rr[:], onerow_sb[:], rs8[0:1, nsl],
                        start=True, stop=True,
                    )
                    nc.vector.tensor_copy(out=rsb[:, nsl], in_=rr[:])
                # scaled-U in fp8: uts = ut * rs8 (per-row) / 8
                for kc in range(2):
                    nc.vector.scalar_tensor_tensor(
                        out=uts[:, kc, :], in0=ut[:, kc, :], scalar=1.0,
                        in1=rsb[:], op0=MULT, op1=MULT,
                    )

                # ---- per row-block: logits+lnC matmul -> exp -> reduce ---
                for rb in range(NBLK):
                    col = i * NBLK + rb
                    ph = col % 2
                    if ph == 0:
                        m_sb = m_pool.tile([P, 2, BT], FP8, tag="m")
                        nc.sync.dma_start(out=m_sb[:], in_=mln_d[:, col // 2])
                    lhs = uts[:, :, rb * P:(rb + 1) * P]
                    for pt_i in range(NPT):
                        ps = psum_s.tile([P, PTILE], F32, tag="ps")
                        for s in range(PTILE // 512):
                            csl = slice(
                                pt_i * PTILE + s * 512, pt_i * PTILE + (s + 1) * 512
                            )
                            psl = slice(s * 512, (s + 1) * 512)
                            nc.tensor.matmul(
                                ps[:, psl], lhs, azt_sb[:, :, csl],
                                start=True, stop=False, perf_mode=DR,
                            )
                            nc.tensor.matmul(
                                ps[:, psl], id8_sb[:, ph], m_sb[:, :, csl],
                                start=False, stop=True, perf_mode=DR,
                            )
                        je = junk.tile([P, PTILE], BF16, tag="je")
                        nc.scalar.activation(
                            out=je[:], in_=ps[:],
                            func=mybir.ActivationFunctionType.Exp,
                        )
                        nc.vector.tensor_scalar(
                            out=je[:], in0=je[:], scalar1=1.0, scalar2=0.0,
                            op0=MULT, op1=ADD,
                            accum_out=racc_sb[:, col, pt_i:pt_i + 1],
                        )
            # ---- loss = ln(sum_pt R_pt) - pos_logit ----------------------
            nc.vector.tensor_tensor(
                rsum_sb[:], racc_sb[:, :, 0], racc_sb[:, :, 1], ADD
            )
            for pt_i in range(2, NPT):
                nc.vector.tensor_tensor(
                    rsum_sb[:], rsum_sb[:], racc_sb[:, :, pt_i], ADD
                )
            nc.scalar.activation(
                out=loss_sb[:], in_=rsum_sb[:],
                func=mybir.ActivationFunctionType.Ln,
            )
            nc.vector.tensor_tensor(
                loss_sb[:], loss_sb[:], spT_sb[:], mybir.AluOpType.subtract,
            )

        nc.sync.dma_start(out=loss_d[:], in_=loss_sb[:])

    nc.compile()
    _split_multiwait_drains(nc)
    return nc


def prepare_inputs(z_seq, preds, neg_idx):
    """Host-side sharding/packing. Returns (in_maps, valid_counts)."""
    z_flat = np.asarray(z_seq, dtype=np.float32).reshape(BT, D)
    preds = np.asarray(preds, dtype=np.float32)
    neg_idx = np.asarray(neg_idx)

    norms = np.linalg.norm(z_flat, axis=1, keepdims=True)
    az = z_flat / np.maximum(norms, 1e-12)
    azt8 = np.ascontiguousarray(
        (az.T * FP8_SCALE).reshape(2, P, BT).transpose(1, 0, 2)
    ).astype(E4M3)

    # pt[d, i*4+kc*2+mc, e] = preds[i, mc*128+e, kc*128+d]
    pt = np.empty((P, H * 4, P), dtype=ml_dtypes.bfloat16)
    for i in range(H):
        w = preds[i]  # [e_out, d_in]
        for kc in range(2):
            for mc in range(2):
                blk = w[mc * P:(mc + 1) * P, kc * P:(kc + 1) * P]  # [e, d]
                pt[:, i * 4 + kc * 2 + mc, :] = blk.T.astype(ml_dtypes.bfloat16)

    # DoubleRow identity planes for the M-add matmul
    id8 = np.zeros((P, 2, 2, P), dtype=E4M3)
    eye = np.eye(P, dtype=np.float32).astype(E4M3)
    id8[:, 0, 0, :] = eye
    id8[:, 1, 1, :] = eye

    # ln(count) lookup (counts are small ints; avoid a big np.log)
    lut = np.full(260, NEG_M, dtype=np.float32)
    lut[1:] = np.log(np.arange(1, 260, dtype=np.float32))
    lut[0] = NEG_M

    in_maps = []
    valid_counts = np.zeros((N_CORES, H), dtype=np.int64)
    for c in range(N_CORES):
        n0 = c * NROW
        zat = np.zeros((P, H * 2, NROW), dtype=ml_dtypes.bfloat16)
        azp = np.zeros((P, H * 2, NROW), dtype=ml_dtypes.bfloat16)
        mln = np.zeros((P, NPAIR, 2, BT), dtype=E4M3)
        for i, k in enumerate(HORIZONS):
            L = T - k
            BL = B * L
            nvalid = min(max(BL - n0, 0), NROW)
            valid_counts[c, i] = nvalid
            n = n0 + np.arange(NROW)
            nv = n[:nvalid]
            b = nv // L
            a_full = np.zeros(NROW, dtype=np.int64)
            a_full[:nvalid] = nv + b * k          # anchor flat rows
            p_full = np.zeros(NROW, dtype=np.int64)
            p_full[:nvalid] = nv + (b + 1) * k    # positive flat rows
            zat[:, i * 2:(i + 1) * 2, :] = (
                z_flat[a_full].T.reshape(2, P, NROW).transpose(1, 0, 2)
            ).astype(ml_dtypes.bfloat16)
            azp[:, i * 2:(i + 1) * 2, :] = (
                az[p_full].T.reshape(2, P, NROW).transpose(1, 0, 2)
            ).astype(ml_dtypes.bfloat16)

            # integer counts: negatives multiplicity + 1 at the positive
            cm = np.zeros((NROW, BT), dtype=np.int16)
            rows = np.repeat(np.arange(nvalid), N_NEG)
            np.add.at(cm, (rows, neg_idx[i, nv, :].reshape(-1)), 1)
            cm[np.arange(NROW), p_full] += 1
            if nvalid < NROW:
                # pad rows: single count at slot 0 (host ignores them)
                cm[nvalid:] = 0
                cm[nvalid:, 0] = 1
            mm = lut[cm]  # [NROW, BT] f32: ln(count) or NEG_M
            mm8 = mm.astype(E4M3)
            for rb in range(NBLK):
                col = i * NBLK + rb
                mln[:, col // 2, col % 2, :] = mm8[rb * P:(rb + 1) * P]

        in_maps.append(
            {"azt": azt8, "zat": zat, "azp": azp, "pt": pt, "id8": id8, "mln": mln}
        )
    return in_maps, valid_counts


def reduce_outputs(results, valid_counts):
    raw_w = {k: 1.0 / math.sqrt(k) for k in HORIZONS}
    tot_w = sum(raw_w.values())
    total = np.float64(0.0)
    for i, k in enumerate(HORIZONS):
        L = T - k
        BL = B * L
        s = np.float64(0.0)
        for c in range(N_CORES):
            nvalid = int(valid_counts[c, i])
            if nvalid == 0:
                continue
            lm = results[c]["loss"]  # [P, NCOL]
            per_row = lm[:, i * NBLK:(i + 1) * NBLK].T.reshape(NROW)
            s += per_row[:nvalid].sum(dtype=np.float64)
        total += (raw_w[k] / tot_w) * (s / BL)
    return np.float32(total)


_CACHED_NC = None


def kernel(z_seq, preds, neg_idx):
    global _CACHED_NC
    if _CACHED_NC is None:
        _CACHED_NC = build_program()
    nc = _CACHED_NC
    in_maps, valid_counts = prepare_inputs(z_seq, preds, neg_idx)
    res = run_bass_kernel_spmd(nc, in_maps, list(range(N_CORES)))
    return reduce_outputs(res.results, valid_counts)


if __name__ == "__main__":
    rng = np.random.default_rng(0)
    z = rng.standard_normal((B, T, D), dtype=np.float32)
    pr = (rng.standard_normal((H, D, D), dtype=np.float32) / np.sqrt(D)).astype(
        np.float32
    )
    ni = rng.integers(0, BT, size=(H, BT, N_NEG), dtype=np.int64)
    print(kernel(z, pr, ni))


# revision 10
# speedup vs baseline: 1.1731x; 1.1102x over previous
"""CPC InfoNCE loss kernel for Trainium2 (8 NeuronCores, data-parallel rows).

Per core (rows sharded across cores, 3 horizons x 8 blocks of 128 rows):
  - Host normalizes the pool table all_z = normalize(z_seq.reshape(BT, D)),
    scales it by 8 and uploads it transposed in fp8-e4m3 (AZT, DoubleRow kc
    planes). Host also gathers per-core anchor and positive rows (transposed,
    bf16), the predictor weights (transposed, bf16), and a dense log-count
    matrix M [row, pool] in fp8 (ln of the multiplicity of each pool entry
    among the row's sampled negatives + positive; -104 where unsampled, so
    exp maps it to ~0).
  - PE computes U^T = W @ Z_anchor^T (bf16); per-row norms ||u||^2 and raw
    positive dots via ones-matmuls; rs8 = 1/(8*tau*||u||) per row.
  - DVE scales U by rs8 (broadcast via a PE ones-outer-product) into fp8.
    The scaled-U fp8 DoubleRow matmul against AZT then produces the final
    logits directly in PSUM, and a second fp8 DoubleRow matmul with an
    identity lhsT accumulates M on top: PSUM = logit + ln(count).
  - ACT applies a plain Exp out of PSUM (bf16 out); DVE reduces each row
    with a tensor_scalar accumulate: R = sum_m count*e^logit per row-block.
  - loss = ln(R) - pos_logit per row; host averages with horizon weights.
"""

import sys

sys.path.insert(0, "/opt/trn_rl_repo")

import math
import os

import ml_dtypes
import numpy as np

import concourse.bass as bass
import concourse.tile as tile
from concourse import bacc
from concourse import mybir
from concourse.bass_utils import run_bass_kernel_spmd

# Problem constants (hardcoded per contract)
B, T, D = 16, 512, 256
BT = B * T  # 8192 pool entries
HORIZONS = (1, 5, 21)
H = len(HORIZONS)
N_NEG = 128
TAU = 0.07
N_CORES = 8

P = 128
NROW = 1024  # padded rows per core per horizon
NBLK = NROW // P  # 8
NCOL = H * NBLK  # 24 row-blocks per core
NPAIR = NCOL // 2  # 12 block-pairs (DoubleRow M planes)
PTILE = 1024  # pool columns per PSUM tile (2 banks)
NPT = BT // PTILE  # 8
NEG_M = -104.0  # exp(logit + NEG_M) == 0 for unsampled entries
FP8_SCALE = 8.0  # az stored as az*8; u scaled by rs/8

BF16 = mybir.dt.bfloat16
F32 = mybir.dt.float32
FP8 = mybir.dt.float8e4
E4M3 = ml_dtypes.float8_e4m3
MULT = mybir.AluOpType.mult
ADD = mybir.AluOpType.add
DR = mybir.MatmulPerfMode.DoubleRow


def _split_multiwait_drains(nc):
    """This walrus build accepts only one sync-wait command per TPB_CTRL
    instruction; TileContext's exit drain carries one wait per live proc.
    Split the extras into preceding single-wait drains."""
    for f in nc.m.functions:
        for bb in f.blocks:
            new_list = []
            for inst in bb.instructions:
                si = inst.sync_info
                if si is not None and si.on_wait and len(si.on_wait) > 1:
                    waits = list(si.on_wait)
                    for j, w in enumerate(waits[:-1]):
                        d = mybir.InstDrain(
                            name=f"{inst.name}-w{j}", ins=[], outs=[]
                        )
                        d.engine = inst.engine
                        d.sync_info = mybir.SyncInfo(on_wait=[w], on_update=[])
                        nc.register_instruction(d)
                        new_list.append(d)
                    si.on_wait = [waits[-1]]
                    inst.sync_info = si
                new_list.append(inst)
            bb.instructions[:] = new_list


def build_program(reps=1):
    reps = int(os.environ.get("KERNEL_REPS", reps))
    nc = bacc.Bacc(
        "TRN2", target_bir_lowering=False, debug=False, num_devices=N_CORES
    )

    azt_d = nc.declare_dram_parameter("azt", [P, 2, BT], FP8, isOutput=False)
    zat_d = nc.declare_dram_parameter("zat", [P, H * 2, NROW], BF16, isOutput=False)
    azp_d = nc.declare_dram_parameter("azp", [P, H * 2, NROW], BF16, isOutput=False)
    pt_d = nc.declare_dram_parameter("pt", [P, H * 4, P], BF16, isOutput=False)
    id8_d = nc.declare_dram_parameter("id8", [P, 2, 2, P], FP8, isOutput=False)
    mln_d = nc.declare_dram_parameter("mln", [P, NPAIR, 2, BT], FP8, isOutput=False)
    loss_d = nc.declare_dram_parameter("loss", [P, NCOL], F32, isOutput=True)

    from contextlib import ExitStack, nullcontext

    with tile.TileContext(nc) as tc, ExitStack() as ctx:
        singles = ctx.enter_context(tc.tile_pool(name="singles", bufs=1))
        work = ctx.enter_context(tc.tile_pool(name="work", bufs=2))
        m_pool = ctx.enter_context(tc.tile_pool(name="m", bufs=2))
        flats = ctx.enter_context(tc.tile_pool(name="flats", bufs=2))
        junk = ctx.enter_context(tc.tile_pool(name="junk", bufs=2))
        psum_s = ctx.enter_context(tc.tile_pool(name="psum_s", bufs=3, space="PSUM"))
        psum_u = ctx.enter_context(tc.tile_pool(name="psum_u", bufs=1, space="PSUM"))
        psum_sm = ctx.enter_context(tc.tile_pool(name="psum_sm", bufs=1, space="PSUM"))

        # ---- preload constants -------------------------------------------
        azt_sb = singles.tile([P, 2, BT], FP8)
        nc.sync.dma_start(out=azt_sb[:], in_=azt_d[:])
        zat_sb = singles.tile([P, H * 2, NROW], BF16)
        nc.sync.dma_start(out=zat_sb[:], in_=zat_d[:])
        azp_sb = singles.tile([P, H * 2, NROW], BF16)
        nc.sync.dma_start(out=azp_sb[:], in_=azp_d[:])
        pt_sb = singles.tile([P, H * 4, P], BF16)
        nc.sync.dma_start(out=pt_sb[:], in_=pt_d[:])
        id8_sb = singles.tile([P, 2, 2, P], FP8)
        nc.sync.dma_start(out=id8_sb[:], in_=id8_d[:])

        ones_sb = singles.tile([P, 1], BF16)
        nc.vector.memset(ones_sb[:], 1.0)
        onerow_sb = singles.tile([1, P], F32)
        nc.vector.memset(onerow_sb[:], 1.0)
        one1_sb = singles.tile([1, 1], F32)
        nc.vector.memset(one1_sb[:], 1.0)

        loss_sb = singles.tile([P, NCOL], F32)
        rsum_sb = singles.tile([P, NCOL], F32)
        spT_sb = singles.tile([P, NCOL], F32)

        loop_cm = tc.For_i(0, reps, 1) if reps > 1 else nullcontext()
        with loop_cm:
            for i in range(H):
                ut = work.tile([P, 2, NROW], BF16, tag="ut")
                uts = work.tile([P, 2, NROW], FP8, tag="uts")
                rsb = work.tile([P, NROW], BF16, tag="rsb")
                rs8 = flats.tile([1, NROW], F32, tag="rs")
                spf = flats.tile([1, NROW], F32, tag="sp")
                nsum = flats.tile([1, NROW], F32, tag="ns")

                # ---- U^T = W @ Z_anchor^T (bf16), copied out by DVE ------
                for mc in range(2):
                    for nh in range(2):
                        nsl = slice(nh * (NROW // 2), (nh + 1) * (NROW // 2))
                        pu = psum_u.tile([P, NROW // 2], F32, tag="pu")
                        for kc in range(2):
                            nc.tensor.matmul(
                                pu[:],
                                pt_sb[:, i * 4 + kc * 2 + mc, :],
                                zat_sb[:, i * 2 + kc, nsl],
                                start=(kc == 0),
                                stop=(kc == 1),
                            )
                        nc.vector.tensor_copy(out=ut[:, mc, nsl], in_=pu[:])

                # ---- per-row ||u||^2 and raw positive dot ----------------
                usq = []
                upr = []
                for mc in range(2):
                    sq = junk.tile([P, NROW], BF16, tag=f"usq{mc}")
                    nc.vector.tensor_mul(sq[:], ut[:, mc, :], ut[:, mc, :])
                    usq.append(sq)
                    pr_ = junk.tile([P, NROW], BF16, tag=f"upr{mc}")
                    nc.vector.tensor_mul(
                        pr_[:], ut[:, mc, :], azp_sb[:, i * 2 + mc, :]
                    )
                    upr.append(pr_)
                for nh in range(2):
                    nsl = slice(nh * (NROW // 2), (nh + 1) * (NROW // 2))
                    pbn = psum_sm.tile([1, NROW // 2], F32, tag="sm")
                    for mc in range(2):
                        nc.tensor.matmul(
                            pbn[:], ones_sb[:], usq[mc][:, nsl],
                            start=(mc == 0), stop=(mc == 1),
                        )
                    nc.vector.tensor_copy(out=nsum[0:1, nsl], in_=pbn[:])
                    pbp = psum_sm.tile([1, NROW // 2], F32, tag="sm")
                    for mc in range(2):
                        nc.tensor.matmul(
                            pbp[:], ones_sb[:], upr[mc][:, nsl],
                            start=(mc == 0), stop=(mc == 1),
                        )
                    nc.vector.tensor_copy(out=spf[0:1, nsl], in_=pbp[:])

                # rs8 = 1/(8*tau*||u||) = 1/sqrt(64*tau^2*||u||^2)
                nc.scalar.activation(
                    out=rs8[:], in_=nsum[:],
                    func=mybir.ActivationFunctionType.Sqrt,
                    scale=float(64.0 * TAU * TAU),
                )
                nc.vector.reciprocal_approx_fast(out=rs8[:], in_=rs8[:])
                # positive logit = raw_dot * rs8 * 8
                nc.vector.scalar_tensor_tensor(
                    out=spf[:], in0=spf[:], scalar=8.0, in1=rs8[:],
                    op0=MULT, op1=MULT,
                )
                # transpose positive logits into per-block columns
                for rb in range(NBLK):
                    col = i * NBLK + rb
                    pr = psum_sm.tile([P, 2], F32, tag="sm")
                    nc.tensor.matmul(
                        pr[:, 0:1], spf[0:1, rb * P:(rb + 1) * P],
                        one1_sb[:], start=True, stop=True,
                    )
                    nc.vector.tensor_copy(out=spT_sb[:, col:col + 1], in_=pr[:, 0:1])

                # broadcast rs8 across partitions -> bf16 [P, NROW]
                for nh in range(2):
                    nsl = slice(nh * (NROW // 2), (nh + 1) * (NROW // 2))
                    rr = psum_sm.tile([P, NROW // 2], F32, tag="sm")
                    nc.tensor.matmul(
                        rr[:], onerow_sb[:], rs8[0:1, nsl],
                        start=True, stop=True,
                    )
                    nc.vector.tensor_copy(out=rsb[:, nsl], in_=rr[:])
                # scaled-U in fp8: uts = ut * rs8 (per-row) / 8
                for kc in range(2):
                    nc.vector.scalar_tensor_tensor(
                        out=uts[:, kc, :], in0=ut[:, kc, :], scalar=1.0,
                        in1=rsb[:], op0=MULT, op1=MULT,
                    )

                # ---- per row-block: logits+lnC matmul -> exp -> reduce ---
                for rb in range(NBLK):
                    col = i * NBLK + rb
                    ph = col % 2
                    if ph == 0:
                        m_sb = m_pool.tile([P, 2, BT], FP8, tag="m")
                        nc.sync.dma_start(out=m_sb[:], in_=mln_d[:, col // 2])
                    lhs = uts[:, :, rb * P:(rb + 1) * P]
                    je = junk.tile([P, BT], BF16, tag="je")
                    for pt_i in range(NPT):
                        ps = psum_s.tile([P, PTILE], F32, tag="ps")
                        for s in range(PTILE // 512):
                            csl = slice(
                                pt_i * PTILE + s * 512, pt_i * PTILE + (s + 1) * 512
                            )
                            psl = slice(s * 512, (s + 1) * 512)
                            nc.tensor.matmul(
                                ps[:, psl], lhs, azt_sb[:, :, csl],
                                start=True, stop=False, perf_mode=DR,
                            )
                            nc.tensor.matmul(
                                ps[:, psl], id8_sb[:, ph], m_sb[:, :, csl],
                                start=False, stop=True, perf_mode=DR,
                            )
                        nc.scalar.activation(
                            out=je[:, pt_i * PTILE:(pt_i + 1) * PTILE], in_=ps[:],
                            func=mybir.ActivationFunctionType.Exp,
                        )
                    nc.vector.tensor_scalar(
                        out=je[:], in0=je[:], scalar1=1.0, scalar2=0.0,
                        op0=MULT, op1=ADD,
                        accum_out=rsum_sb[:, col:col + 1],
                    )
            # ---- loss = ln(R) - pos_logit --------------------------------
            nc.scalar.activation(
                out=loss_sb[:], in_=rsum_sb[:],
                func=mybir.ActivationFunctionType.Ln,
            )
            nc.vector.tensor_tensor(
                loss_sb[:], loss_sb[:], spT_sb[:], mybir.AluOpType.subtract,
            )

        nc.sync.dma_start(out=loss_d[:], in_=loss_sb[:])

    nc.compile()
    _split_multiwait_drains(nc)
    return nc


def prepare_inputs(z_seq, preds, neg_idx):
    """Host-side sharding/packing. Returns (in_maps, valid_counts)."""
    z_flat = np.asarray(z_seq, dtype=np.float32).reshape(BT, D)
    preds = np.asarray(preds, dtype=np.float32)
    neg_idx = np.asarray(neg_idx)

    norms = np.linalg.norm(z_flat, axis=1, keepdims=True)
    az = z_flat / np.maximum(norms, 1e-12)
    azt8 = np.ascontiguousarray(
        (az.T * FP8_SCALE).reshape(2, P, BT).transpose(1, 0, 2)
    ).astype(E4M3)

    # pt[d, i*4+kc*2+mc, e] = preds[i, mc*128+e, kc*128+d]
    pt = np.empty((P, H * 4, P), dtype=ml_dtypes.bfloat16)
    for i in range(H):
        w = preds[i]  # [e_out, d_in]
        for kc in range(2):
            for mc in range(2):
                blk = w[mc * P:(mc + 1) * P, kc * P:(kc + 1) * P]  # [e, d]
                pt[:, i * 4 + kc * 2 + mc, :] = blk.T.astype(ml_dtypes.bfloat16)

    # DoubleRow identity planes for the M-add matmul
    id8 = np.zeros((P, 2, 2, P), dtype=E4M3)
    eye = np.eye(P, dtype=np.float32).astype(E4M3)
    id8[:, 0, 0, :] = eye
    id8[:, 1, 1, :] = eye

    # ln(count) lookup (counts are small ints; avoid a big np.log)
    lut = np.full(260, NEG_M, dtype=np.float32)
    lut[1:] = np.log(np.arange(1, 260, dtype=np.float32))
    lut[0] = NEG_M

    in_maps = []
    valid_counts = np.zeros((N_CORES, H), dtype=np.int64)
    for c in range(N_CORES):
        n0 = c * NROW
        zat = np.zeros((P, H * 2, NROW), dtype=ml_dtypes.bfloat16)
        azp = np.zeros((P, H * 2, NROW), dtype=ml_dtypes.bfloat16)
        mln = np.zeros((P, NPAIR, 2, BT), dtype=E4M3)
        for i, k in enumerate(HORIZONS):
            L = T - k
            BL = B * L
            nvalid = min(max(BL - n0, 0), NROW)
            valid_counts[c, i] = nvalid
            n = n0 + np.arange(NROW)
            nv = n[:nvalid]
            b = nv // L
            a_full = np.zeros(NROW, dtype=np.int64)
            a_full[:nvalid] = nv + b * k          # anchor flat rows
            p_full = np.zeros(NROW, dtype=np.int64)
            p_full[:nvalid] = nv + (b + 1) * k    # positive flat rows
            zat[:, i * 2:(i + 1) * 2, :] = (
                z_flat[a_full].T.reshape(2, P, NROW).transpose(1, 0, 2)
            ).astype(ml_dtypes.bfloat16)
            azp[:, i * 2:(i + 1) * 2, :] = (
                az[p_full].T.reshape(2, P, NROW).transpose(1, 0, 2)
            ).astype(ml_dtypes.bfloat16)

            # integer counts: negatives multiplicity + 1 at the positive
            cm = np.zeros((NROW, BT), dtype=np.int16)
            rows = np.repeat(np.arange(nvalid), N_NEG)
            np.add.at(cm, (rows, neg_idx[i, nv, :].reshape(-1)), 1)
            cm[np.arange(NROW), p_full] += 1
            if nvalid < NROW:
                # pad rows: single count at slot 0 (host ignores them)
                cm[nvalid:] = 0
                cm[nvalid:, 0] = 1
            mm = lut[cm]  # [NROW, BT] f32: ln(count) or NEG_M
            mm8 = mm.astype(E4M3)
            for rb in range(NBLK):
                col = i * NBLK + rb
                mln[:, col // 2, col % 2, :] = mm8[rb * P:(rb + 1) * P]

        in_maps.append(
            {"azt": azt8, "zat": zat, "azp": azp, "pt": pt, "id8": id8, "mln": mln}
        )
    return in_maps, valid_counts


def reduce_outputs(results, valid_counts):
    raw_w = {k: 1.0 / math.sqrt(k) for k in HORIZONS}
    tot_w = sum(raw_w.values())
    total = np.float64(0.0)
    for i, k in enumerate(HORIZONS):
        L = T - k
        BL = B * L
        s = np.float64(0.0)
        for c in range(N_CORES):
            nvalid = int(valid_counts[c, i])
            if nvalid == 0:
                continue
            lm = results[c]["loss"]  # [P, NCOL]
            per_row = lm[:, i * NBLK:(i + 1) * NBLK].T.reshape(NROW)
            s += per_row[:nvalid].sum(dtype=np.float64)
        total += (raw_w[k] / tot_w) * (s / BL)
    return np.float32(total)


_CACHED_NC = None


def kernel(z_seq, preds, neg_idx):
    global _CACHED_NC
    if _CACHED_NC is None:
        _CACHED_NC = build_program()
    nc = _CACHED_NC
    in_maps, valid_counts = prepare_inputs(z_seq, preds, neg_idx)
    res = run_bass_kernel_spmd(nc, in_maps, list(range(N_CORES)))
    return reduce_outputs(res.results, valid_counts)


if __name__ == "__main__":
    rng = np.random.default_rng(0)
    z = rng.standard_normal((B, T, D), dtype=np.float32)
    pr = (rng.standard_normal((H, D, D), dtype=np.float32) / np.sqrt(D)).astype(
        np.float32
    )
    ni = rng.integers(0, BT, size=(H, BT, N_NEG), dtype=np.int64)
    print(kernel(z, pr, ni))


# revision 12
# speedup vs baseline: 1.2467x; 1.0628x over previous
"""CPC InfoNCE loss kernel for Trainium2 (8 NeuronCores, data-parallel rows).

Per core (rows sharded across cores, 3 horizons x 8 blocks of 128 rows):
  - Host normalizes the pool table all_z = normalize(z_seq.reshape(BT, D)),
    scales it by 8 and uploads it transposed in fp8-e4m3 (AZT, DoubleRow kc
    planes). Host also gathers per-core anchor and positive rows (transposed,
    bf16), the predictor weights (transposed, bf16), and a dense log-count
    matrix M [row, pool] in fp8 (ln of the multiplicity of each pool entry
    among the row's sampled negatives + positive; -104 where unsampled, so
    exp maps it to ~0).
  - PE computes U^T = W @ Z_anchor^T (bf16); per-row norms ||u||^2 and raw
    positive dots via ones-matmuls; rs8 = 1/(8*tau*||u||) per row.
  - DVE scales U by rs8 (broadcast via a PE ones-outer-product) into fp8.
    The scaled-U fp8 DoubleRow matmul against AZT then produces the final
    logits directly in PSUM, and a second fp8 DoubleRow matmul with an
    identity lhsT accumulates M on top: PSUM = logit + ln(count).
  - ACT applies a plain Exp out of PSUM (bf16 out); DVE reduces each row
    with a tensor_scalar accumulate: R = sum_m count*e^logit per row-block.
  - loss = ln(R) - pos_logit per row; host averages with horizon weights.
"""

import sys

sys.path.insert(0, "/opt/trn_rl_repo")

import math
import os

import ml_dtypes
import numpy as np

import concourse.bass as bass
import concourse.tile as tile
from concourse import bacc
from concourse import mybir
from concourse.bass_utils import run_bass_kernel_spmd

# Problem constants (hardcoded per contract)
B, T, D = 16, 512, 256
BT = B * T  # 8192 pool entries
HORIZONS = (1, 5, 21)
H = len(HORIZONS)
N_NEG = 128
TAU = 0.07
N_CORES = 8

P = 128
NROW = 1024  # padded rows per core per horizon
NBLK = NROW // P  # 8
NCOL = H * NBLK  # 24 row-blocks per core
NPAIR = NCOL // 2  # 12 block-pairs (DoubleRow M planes)
PTILE = 2048  # pool columns per PSUM tile (4 banks)
NPT = BT // PTILE  # 4
NEG_M = -104.0  # exp(logit + NEG_M) == 0 for unsampled entries
FP8_SCALE = 8.0  # az stored as az*8; u scaled by rs/8

BF16 = mybir.dt.bfloat16
F32 = mybir.dt.float32
FP8 = mybir.dt.float8e4
E4M3 = ml_dtypes.float8_e4m3
MULT = mybir.AluOpType.mult
ADD = mybir.AluOpType.add
DR = mybir.MatmulPerfMode.DoubleRow


def _split_multiwait_drains(nc):
    """This walrus build accepts only one sync-wait command per TPB_CTRL
    instruction; TileContext's exit drain carries one wait per live proc.
    Split the extras into preceding single-wait drains."""
    for f in nc.m.functions:
        for bb in f.blocks:
            new_list = []
            for inst in bb.instructions:
                si = inst.sync_info
                if si is not None and si.on_wait and len(si.on_wait) > 1:
                    waits = list(si.on_wait)
                    for j, w in enumerate(waits[:-1]):
                        d = mybir.InstDrain(
                            name=f"{inst.name}-w{j}", ins=[], outs=[]
                        )
                        d.engine = inst.engine
                        d.sync_info = mybir.SyncInfo(on_wait=[w], on_update=[])
                        nc.register_instruction(d)
                        new_list.append(d)
                    si.on_wait = [waits[-1]]
                    inst.sync_info = si
                new_list.append(inst)
            bb.instructions[:] = new_list


def build_program(reps=1):
    reps = int(os.environ.get("KERNEL_REPS", reps))
    nc = bacc.Bacc(
        "TRN2", target_bir_lowering=False, debug=False, num_devices=N_CORES
    )

    azt_d = nc.declare_dram_parameter("azt", [P, 2, BT], FP8, isOutput=False)
    zat_d = nc.declare_dram_parameter("zat", [P, H * 2, NROW], BF16, isOutput=False)
    azp_d = nc.declare_dram_parameter("azp", [P, H * 2, NROW], BF16, isOutput=False)
    pt_d = nc.declare_dram_parameter("pt", [P, H * 4, P], BF16, isOutput=False)
    id8_d = nc.declare_dram_parameter("id8", [P, 2, 2, P], FP8, isOutput=False)
    mln_d = nc.declare_dram_parameter("mln", [P, NPAIR, 2, BT], FP8, isOutput=False)
    loss_d = nc.declare_dram_parameter("loss", [P, NCOL], F32, isOutput=True)

    from contextlib import ExitStack, nullcontext

    with tile.TileContext(nc) as tc, ExitStack() as ctx:
        singles = ctx.enter_context(tc.tile_pool(name="singles", bufs=1))
        work = ctx.enter_context(tc.tile_pool(name="work", bufs=2))
        m_pool = ctx.enter_context(tc.tile_pool(name="m", bufs=2))
        flats = ctx.enter_context(tc.tile_pool(name="flats", bufs=2))
        junk = ctx.enter_context(tc.tile_pool(name="junk", bufs=2))
        psum_s = ctx.enter_context(tc.tile_pool(name="psum_s", bufs=2, space="PSUM"))
        psum_u = psum_s
        psum_sm = psum_s

        # ---- preload constants -------------------------------------------
        azt_sb = singles.tile([P, 2, BT], FP8)
        nc.sync.dma_start(out=azt_sb[:], in_=azt_d[:])
        zat_sb = singles.tile([P, H * 2, NROW], BF16)
        nc.sync.dma_start(out=zat_sb[:], in_=zat_d[:])
        azp_sb = singles.tile([P, H * 2, NROW], BF16)
        nc.sync.dma_start(out=azp_sb[:], in_=azp_d[:])
        pt_sb = singles.tile([P, H * 4, P], BF16)
        nc.sync.dma_start(out=pt_sb[:], in_=pt_d[:])
        id8_sb = singles.tile([P, 2, 2, P], FP8)
        nc.sync.dma_start(out=id8_sb[:], in_=id8_d[:])

        ones_sb = singles.tile([P, 1], BF16)
        nc.vector.memset(ones_sb[:], 1.0)
        onerow_sb = singles.tile([1, P], F32)
        nc.vector.memset(onerow_sb[:], 1.0)
        one1_sb = singles.tile([1, 1], F32)
        nc.vector.memset(one1_sb[:], 1.0)

        loss_sb = singles.tile([P, NCOL], F32)
        rsum_sb = singles.tile([P, NCOL], F32)
        spT_sb = singles.tile([P, NCOL], F32)
        racc_sb = singles.tile([P, NCOL, NPT], F32)

        loop_cm = tc.For_i(0, reps, 1) if reps > 1 else nullcontext()
        with loop_cm:
            for i in range(H):
                ut = work.tile([P, 2, NROW], BF16, tag="ut")
                uts = work.tile([P, 2, NROW], FP8, tag="uts")
                rsb = work.tile([P, NROW], BF16, tag="rsb")
                rs8 = flats.tile([1, NROW], F32, tag="rs")
                spf = flats.tile([1, NROW], F32, tag="sp")
                nsum = flats.tile([1, NROW], F32, tag="ns")

                # ---- U^T = W @ Z_anchor^T (bf16), copied out by DVE ------
                for mc in range(2):
                    for nh in range(2):
                        nsl = slice(nh * (NROW // 2), (nh + 1) * (NROW // 2))
                        pu = psum_u.tile([P, NROW // 2], F32, tag="ps")
                        for kc in range(2):
                            nc.tensor.matmul(
                                pu[:],
                                pt_sb[:, i * 4 + kc * 2 + mc, :],
                                zat_sb[:, i * 2 + kc, nsl],
                                start=(kc == 0),
                                stop=(kc == 1),
                            )
                        nc.vector.tensor_copy(out=ut[:, mc, nsl], in_=pu[:])

                # ---- per-row ||u||^2 and raw positive dot ----------------
                usq = []
                upr = []
                for mc in range(2):
                    sq = junk.tile([P, NROW], BF16, tag=f"usq{mc}")
                    nc.vector.tensor_mul(sq[:], ut[:, mc, :], ut[:, mc, :])
                    usq.append(sq)
                    pr_ = junk.tile([P, NROW], BF16, tag=f"upr{mc}")
                    nc.vector.tensor_mul(
                        pr_[:], ut[:, mc, :], azp_sb[:, i * 2 + mc, :]
                    )
                    upr.append(pr_)
                for nh in range(2):
                    nsl = slice(nh * (NROW // 2), (nh + 1) * (NROW // 2))
                    pbn = psum_sm.tile([1, NROW // 2], F32, tag="ps")
                    for mc in range(2):
                        nc.tensor.matmul(
                            pbn[:], ones_sb[:], usq[mc][:, nsl],
                            start=(mc == 0), stop=(mc == 1),
                        )
                    nc.vector.tensor_copy(out=nsum[0:1, nsl], in_=pbn[:])
                    pbp = psum_sm.tile([1, NROW // 2], F32, tag="ps")
                    for mc in range(2):
                        nc.tensor.matmul(
                            pbp[:], ones_sb[:], upr[mc][:, nsl],
                            start=(mc == 0), stop=(mc == 1),
                        )
                    nc.vector.tensor_copy(out=spf[0:1, nsl], in_=pbp[:])

                # rs8 = 1/(8*tau*||u||) = 1/sqrt(64*tau^2*||u||^2)
                nc.scalar.activation(
                    out=rs8[:], in_=nsum[:],
                    func=mybir.ActivationFunctionType.Sqrt,
                    scale=float(64.0 * TAU * TAU),
                )
                nc.vector.reciprocal_approx_fast(out=rs8[:], in_=rs8[:])
                # positive logit = raw_dot * rs8 * 8
                nc.vector.scalar_tensor_tensor(
                    out=spf[:], in0=spf[:], scalar=8.0, in1=rs8[:],
                    op0=MULT, op1=MULT,
                )
                # transpose positive logits into per-block columns
                for rb in range(NBLK):
                    col = i * NBLK + rb
                    pr = psum_sm.tile([P, 2], F32, tag="ps")
                    nc.tensor.matmul(
                        pr[:, 0:1], spf[0:1, rb * P:(rb + 1) * P],
                        one1_sb[:], start=True, stop=True,
                    )
                    nc.vector.tensor_copy(out=spT_sb[:, col:col + 1], in_=pr[:, 0:1])

                # broadcast rs8 across partitions -> bf16 [P, NROW]
                for nh in range(2):
                    nsl = slice(nh * (NROW // 2), (nh + 1) * (NROW // 2))
                    rr = psum_sm.tile([P, NROW // 2], F32, tag="ps")
                    nc.tensor.matmul(
                        rr[:], onerow_sb[:], rs8[0:1, nsl],
                        start=True, stop=True,
                    )
                    nc.vector.tensor_copy(out=rsb[:, nsl], in_=rr[:])
                # scaled-U in fp8: uts = ut * rs8 (per-row) / 8
                for kc in range(2):
                    nc.vector.scalar_tensor_tensor(
                        out=uts[:, kc, :], in0=ut[:, kc, :], scalar=1.0,
                        in1=rsb[:], op0=MULT, op1=MULT,
                    )

                # ---- per row-block: logits+lnC matmul -> exp -> reduce ---
                for rb in range(NBLK):
                    col = i * NBLK + rb
                    ph = col % 2
                    if ph == 0:
                        m_sb = m_pool.tile([P, 2, BT], FP8, tag="m")
                        nc.sync.dma_start(out=m_sb[:], in_=mln_d[:, col // 2])
                    lhs = uts[:, :, rb * P:(rb + 1) * P]
                    je = junk.tile([P, BT], BF16, tag="je")
                    for pt_i in range(NPT):
                        ps = psum_s.tile([P, PTILE], F32, tag="ps")
                        for s in range(PTILE // 512):
                            csl = slice(
                                pt_i * PTILE + s * 512, pt_i * PTILE + (s + 1) * 512
                            )
                            psl = slice(s * 512, (s + 1) * 512)
                            nc.tensor.matmul(
                                ps[:, psl], lhs, azt_sb[:, :, csl],
                                start=True, stop=False, perf_mode=DR,
                            )
                            nc.tensor.matmul(
                                ps[:, psl], id8_sb[:, ph], m_sb[:, :, csl],
                                start=False, stop=True, perf_mode=DR,
                            )
                        nc.scalar.activation(
                            out=je[:, pt_i * PTILE:(pt_i + 1) * PTILE], in_=ps[:],
                            func=mybir.ActivationFunctionType.Exp,
                            accum_out=racc_sb[:, col, pt_i:pt_i + 1],
                        )
            # ---- loss = ln(sum_pt R_pt) - pos_logit ----------------------
            nc.vector.tensor_tensor(
                rsum_sb[:], racc_sb[:, :, 0], racc_sb[:, :, 1], ADD
            )
            for pt_i in range(2, NPT):
                nc.vector.tensor_tensor(
                    rsum_sb[:], rsum_sb[:], racc_sb[:, :, pt_i], ADD
                )
            nc.scalar.activation(
                out=loss_sb[:], in_=rsum_sb[:],
                func=mybir.ActivationFunctionType.Ln,
            )
            nc.vector.tensor_tensor(
                loss_sb[:], loss_sb[:], spT_sb[:], mybir.AluOpType.subtract,
            )

        nc.sync.dma_start(out=loss_d[:], in_=loss_sb[:])

    nc.compile()
    _split_multiwait_drains(nc)
    return nc


def prepare_inputs(z_seq, preds, neg_idx):
    """Host-side sharding/packing. Returns (in_maps, valid_counts)."""
    z_flat = np.asarray(z_seq, dtype=np.float32).reshape(BT, D)
    preds = np.asarray(preds, dtype=np.float32)
    neg_idx = np.asarray(neg_idx)

    norms = np.linalg.norm(z_flat, axis=1, keepdims=True)
    az = z_flat / np.maximum(norms, 1e-12)
    azt8 = np.ascontiguousarray(
        (az.T * FP8_SCALE).reshape(2, P, BT).transpose(1, 0, 2)
    ).astype(E4M3)

    # pt[d, i*4+kc*2+mc, e] = preds[i, mc*128+e, kc*128+d]
    pt = np.empty((P, H * 4, P), dtype=ml_dtypes.bfloat16)
    for i in range(H):
        w = preds[i]  # [e_out, d_in]
        for kc in range(2):
            for mc in range(2):
                blk = w[mc * P:(mc + 1) * P, kc * P:(kc + 1) * P]  # [e, d]
                pt[:, i * 4 + kc * 2 + mc, :] = blk.T.astype(ml_dtypes.bfloat16)

    # DoubleRow identity planes for the M-add matmul
    id8 = np.zeros((P, 2, 2, P), dtype=E4M3)
    eye = np.eye(P, dtype=np.float32).astype(E4M3)
    id8[:, 0, 0, :] = eye
    id8[:, 1, 1, :] = eye

    # ln(count) lookup (counts are small ints; avoid a big np.log)
    lut = np.full(260, NEG_M, dtype=np.float32)
    lut[1:] = np.log(np.arange(1, 260, dtype=np.float32))
    lut[0] = NEG_M

    in_maps = []
    valid_counts = np.zeros((N_CORES, H), dtype=np.int64)
    for c in range(N_CORES):
        n0 = c * NROW
        zat = np.zeros((P, H * 2, NROW), dtype=ml_dtypes.bfloat16)
        azp = np.zeros((P, H * 2, NROW), dtype=ml_dtypes.bfloat16)
        mln = np.zeros((P, NPAIR, 2, BT), dtype=E4M3)
        for i, k in enumerate(HORIZONS):
            L = T - k
            BL = B * L
            nvalid = min(max(BL - n0, 0), NROW)
            valid_counts[c, i] = nvalid
            n = n0 + np.arange(NROW)
            nv = n[:nvalid]
            b = nv // L
            a_full = np.zeros(NROW, dtype=np.int64)
            a_full[:nvalid] = nv + b * k          # anchor flat rows
            p_full = np.zeros(NROW, dtype=np.int64)
            p_full[:nvalid] = nv + (b + 1) * k    # positive flat rows
            zat[:, i * 2:(i + 1) * 2, :] = (
                z_flat[a_full].T.reshape(2, P, NROW).transpose(1, 0, 2)
            ).astype(ml_dtypes.bfloat16)
            azp[:, i * 2:(i + 1) * 2, :] = (
                az[p_full].T.reshape(2, P, NROW).transpose(1, 0, 2)
            ).astype(ml_dtypes.bfloat16)

            # integer counts: negatives multiplicity + 1 at the positive
            cm = np.zeros((NROW, BT), dtype=np.int16)
            rows = np.repeat(np.arange(nvalid), N_NEG)
            np.add.at(cm, (rows, neg_idx[i, nv, :].reshape(-1)), 1)
            cm[np.arange(NROW), p_full] += 1
            if nvalid < NROW:
                # pad rows: single count at slot 0 (host ignores them)
                cm[nvalid:] = 0
                cm[nvalid:, 0] = 1
            mm = lut[cm]  # [NROW, BT] f32: ln(count) or NEG_M
            mm8 = mm.astype(E4M3)
            for rb in range(NBLK):
                col = i * NBLK + rb
                mln[:, col // 2, col % 2, :] = mm8[rb * P:(rb + 1) * P]

        in_maps.append(
            {"azt": azt8, "zat": zat, "azp": azp, "pt": pt, "id8": id8, "mln": mln}
        )
    return in_maps, valid_counts


def reduce_outputs(results, valid_counts):
    raw_w = {k: 1.0 / math.sqrt(k) for k in HORIZONS}
    tot_w = sum(raw_w.values())
    total = np.float64(0.0)
    for i, k in enumerate(HORIZONS):
        L = T - k
        BL = B * L
        s = np.float64(0.0)
        for c in range(N_CORES):
            nvalid = int(valid_counts[c, i])
            if nvalid == 0:
                continue
            lm = results[c]["loss"]  # [P, NCOL]
            per_row = lm[:, i * NBLK:(i + 1) * NBLK].T.reshape(NROW)
            s += per_row[:nvalid].sum(dtype=np.float64)
        total += (raw_w[k] / tot_w) * (s / BL)
    return np.float32(total)


_CACHED_NC = None


def kernel(z_seq, preds, neg_idx):
    global _CACHED_NC
    if _CACHED_NC is None:
        _CACHED_NC = build_program()
    nc = _CACHED_NC
    in_maps, valid_counts = prepare_inputs(z_seq, preds, neg_idx)
    res = run_bass_kernel_spmd(nc, in_maps, list(range(N_CORES)))
    return reduce_outputs(res.results, valid_counts)


if __name__ == "__main__":
    rng = np.random.default_rng(0)
    z = rng.standard_normal((B, T, D), dtype=np.float32)
    pr = (rng.standard_normal((H, D, D), dtype=np.float32) / np.sqrt(D)).astype(
        np.float32
    )
    ni = rng.integers(0, BT, size=(H, BT, N_NEG), dtype=np.int64)
    print(kernel(z, pr, ni))


# revision 13
# speedup vs baseline: 1.3492x; 1.0822x over previous
"""CPC InfoNCE loss kernel for Trainium2 (8 NeuronCores, data-parallel rows).

Per core (rows sharded across cores, 3 horizons x 8 blocks of 128 rows):
  - Host normalizes the pool table all_z = normalize(z_seq.reshape(BT, D)),
    scales it by 8 and uploads it transposed in fp8-e4m3 (AZT, DoubleRow kc
    planes). Host also gathers per-core anchor and positive rows (transposed,
    bf16), the predictor weights (transposed, bf16), and a dense log-count
    matrix M [row, pool] in fp8 (ln of the multiplicity of each pool entry
    among the row's sampled negatives + positive; -104 where unsampled, so
    exp maps it to ~0).
  - PE computes U^T = W @ Z_anchor^T (bf16); per-row norms ||u||^2 and raw
    positive dots via ones-matmuls; rs8 = 1/(8*tau*||u||) per row.
  - DVE scales U by rs8 (broadcast via a PE ones-outer-product) into fp8.
    The scaled-U fp8 DoubleRow matmul against AZT then produces the final
    logits directly in PSUM, and a second fp8 DoubleRow matmul with an
    identity lhsT accumulates M on top: PSUM = logit + ln(count).
  - ACT applies a plain Exp out of PSUM (bf16 out); DVE reduces each row
    with a tensor_scalar accumulate: R = sum_m count*e^logit per row-block.
  - loss = ln(R) - pos_logit per row; host averages with horizon weights.
"""

import sys

sys.path.insert(0, "/opt/trn_rl_repo")

import math
import os

import ml_dtypes
import numpy as np

import concourse.bass as bass
import concourse.tile as tile
from concourse import bacc
from concourse import mybir
from concourse.bass_utils import run_bass_kernel_spmd

# Problem constants (hardcoded per contract)
B, T, D = 16, 512, 256
BT = B * T  # 8192 pool entries
HORIZONS = (1, 5, 21)
H = len(HORIZONS)
N_NEG = 128
TAU = 0.07
N_CORES = 8

P = 128
NROW = 1024  # padded rows per core per horizon
NBLK = NROW // P  # 8
NCOL = H * NBLK  # 24 row-blocks per core
NPAIR = NCOL // 2  # 12 block-pairs (DoubleRow M planes)
PTILE = 2048  # pool columns per PSUM tile (4 banks)
NPT = BT // PTILE  # 4
NEG_M = -104.0  # exp(logit + NEG_M) == 0 for unsampled entries
FP8_SCALE = 8.0  # az stored as az*8; u scaled by rs/8

BF16 = mybir.dt.bfloat16
F32 = mybir.dt.float32
FP8 = mybir.dt.float8e4
E4M3 = ml_dtypes.float8_e4m3
MULT = mybir.AluOpType.mult
ADD = mybir.AluOpType.add
DR = mybir.MatmulPerfMode.DoubleRow


def _split_multiwait_drains(nc):
    """This walrus build accepts only one sync-wait command per TPB_CTRL
    instruction; TileContext's exit drain carries one wait per live proc.
    Split the extras into preceding single-wait drains."""
    for f in nc.m.functions:
        for bb in f.blocks:
            new_list = []
            for inst in bb.instructions:
                si = inst.sync_info
                if si is not None and si.on_wait and len(si.on_wait) > 1:
                    waits = list(si.on_wait)
                    for j, w in enumerate(waits[:-1]):
                        d = mybir.InstDrain(
                            name=f"{inst.name}-w{j}", ins=[], outs=[]
                        )
                        d.engine = inst.engine
                        d.sync_info = mybir.SyncInfo(on_wait=[w], on_update=[])
                        nc.register_instruction(d)
                        new_list.append(d)
                    si.on_wait = [waits[-1]]
                    inst.sync_info = si
                new_list.append(inst)
            bb.instructions[:] = new_list


def build_program(reps=1):
    reps = int(os.environ.get("KERNEL_REPS", reps))
    nc = bacc.Bacc(
        "TRN2", target_bir_lowering=False, debug=False, num_devices=N_CORES
    )

    azt_d = nc.declare_dram_parameter("azt", [P, 2, BT], FP8, isOutput=False)
    zat_d = nc.declare_dram_parameter("zat", [P, H * 2, NROW], BF16, isOutput=False)
    azp_d = nc.declare_dram_parameter("azp", [P, H * 2, NROW], BF16, isOutput=False)
    pt_d = nc.declare_dram_parameter("pt", [P, H * 4, P], BF16, isOutput=False)
    id8_d = nc.declare_dram_parameter("id8", [P, 2, 2, P], FP8, isOutput=False)
    mln_d = nc.declare_dram_parameter("mln", [P, NPAIR, 2, BT], FP8, isOutput=False)
    loss_d = nc.declare_dram_parameter("loss", [P, NCOL], F32, isOutput=True)

    from contextlib import ExitStack, nullcontext

    with tile.TileContext(nc) as tc, ExitStack() as ctx:
        singles = ctx.enter_context(tc.tile_pool(name="singles", bufs=1))
        work = ctx.enter_context(tc.tile_pool(name="work", bufs=2))
        m_pool = ctx.enter_context(tc.tile_pool(name="m", bufs=2))
        flats = ctx.enter_context(tc.tile_pool(name="flats", bufs=2))
        junk = ctx.enter_context(tc.tile_pool(name="junk", bufs=2))
        psum_s = ctx.enter_context(tc.tile_pool(name="psum_s", bufs=2, space="PSUM"))
        psum_u = psum_s
        psum_sm = psum_s

        # ---- preload constants -------------------------------------------
        azt_sb = singles.tile([P, 2, BT], FP8)
        nc.sync.dma_start(out=azt_sb[:], in_=azt_d[:])
        zat_sb = singles.tile([P, H * 2, NROW], BF16)
        nc.sync.dma_start(out=zat_sb[:], in_=zat_d[:])
        azp_sb = singles.tile([P, H * 2, NROW], BF16)
        nc.sync.dma_start(out=azp_sb[:], in_=azp_d[:])
        pt_sb = singles.tile([P, H * 4, P], BF16)
        nc.sync.dma_start(out=pt_sb[:], in_=pt_d[:])
        id8_sb = singles.tile([P, 2, 2, P], FP8)
        nc.sync.dma_start(out=id8_sb[:], in_=id8_d[:])

        ones_sb = singles.tile([P, 1], BF16)
        nc.vector.memset(ones_sb[:], 1.0)
        onerow_sb = singles.tile([1, P], F32)
        nc.vector.memset(onerow_sb[:], 1.0)
        one1_sb = singles.tile([1, 1], F32)
        nc.vector.memset(one1_sb[:], 1.0)

        loss_sb = singles.tile([P, NCOL], F32)
        rsum_sb = singles.tile([P, NCOL], F32)
        spT_sb = singles.tile([P, NCOL], F32)
        racc_sb = singles.tile([P, NCOL, NPT], F32)

        loop_cm = tc.For_i(0, reps, 1) if reps > 1 else nullcontext()
        with loop_cm:
            uts_h = []
            for i in range(H):
                ut = work.tile([P, 2, NROW], BF16, tag="ut")
                uts = work.tile([P, 2, NROW], FP8, tag="uts", bufs=3)
                uts_h.append(uts)
                rsb = work.tile([P, NROW], BF16, tag="rsb")
                rs8 = flats.tile([1, NROW], F32, tag="rs")
                spf = flats.tile([1, NROW], F32, tag="sp")
                nsum = flats.tile([1, NROW], F32, tag="ns")

                # ---- U^T = W @ Z_anchor^T (bf16), copied out by DVE ------
                for mc in range(2):
                    for nh in range(2):
                        nsl = slice(nh * (NROW // 2), (nh + 1) * (NROW // 2))
                        pu = psum_u.tile([P, NROW // 2], F32, tag="ps")
                        for kc in range(2):
                            nc.tensor.matmul(
                                pu[:],
                                pt_sb[:, i * 4 + kc * 2 + mc, :],
                                zat_sb[:, i * 2 + kc, nsl],
                                start=(kc == 0),
                                stop=(kc == 1),
                            )
                        nc.vector.tensor_copy(out=ut[:, mc, nsl], in_=pu[:])

                # ---- per-row ||u||^2 and raw positive dot ----------------
                usq = []
                upr = []
                for mc in range(2):
                    sq = junk.tile([P, NROW], BF16, tag=f"usq{mc}")
                    nc.vector.tensor_mul(sq[:], ut[:, mc, :], ut[:, mc, :])
                    usq.append(sq)
                    pr_ = junk.tile([P, NROW], BF16, tag=f"upr{mc}")
                    nc.vector.tensor_mul(
                        pr_[:], ut[:, mc, :], azp_sb[:, i * 2 + mc, :]
                    )
                    upr.append(pr_)
                for nh in range(2):
                    nsl = slice(nh * (NROW // 2), (nh + 1) * (NROW // 2))
                    pbn = psum_sm.tile([1, NROW // 2], F32, tag="ps")
                    for mc in range(2):
                        nc.tensor.matmul(
                            pbn[:], ones_sb[:], usq[mc][:, nsl],
                            start=(mc == 0), stop=(mc == 1),
                        )
                    nc.vector.tensor_copy(out=nsum[0:1, nsl], in_=pbn[:])
                    pbp = psum_sm.tile([1, NROW // 2], F32, tag="ps")
                    for mc in range(2):
                        nc.tensor.matmul(
                            pbp[:], ones_sb[:], upr[mc][:, nsl],
                            start=(mc == 0), stop=(mc == 1),
                        )
                    nc.vector.tensor_copy(out=spf[0:1, nsl], in_=pbp[:])

                # rs8 = 1/(8*tau*||u||) = 1/sqrt(64*tau^2*||u||^2)
                nc.scalar.activation(
                    out=rs8[:], in_=nsum[:],
                    func=mybir.ActivationFunctionType.Sqrt,
                    scale=float(64.0 * TAU * TAU),
                )
                nc.vector.reciprocal_approx_fast(out=rs8[:], in_=rs8[:])
                # positive logit = raw_dot * rs8 * 8
                nc.vector.scalar_tensor_tensor(
                    out=spf[:], in0=spf[:], scalar=8.0, in1=rs8[:],
                    op0=MULT, op1=MULT,
                )
                # transpose positive logits into per-block columns
                for rb in range(NBLK):
                    col = i * NBLK + rb
                    pr = psum_sm.tile([P, 2], F32, tag="ps")
                    nc.tensor.matmul(
                        pr[:, 0:1], spf[0:1, rb * P:(rb + 1) * P],
                        one1_sb[:], start=True, stop=True,
                    )
                    nc.vector.tensor_copy(out=spT_sb[:, col:col + 1], in_=pr[:, 0:1])

                # broadcast rs8 across partitions -> bf16 [P, NROW]
                for nh in range(2):
                    nsl = slice(nh * (NROW // 2), (nh + 1) * (NROW // 2))
                    rr = psum_sm.tile([P, NROW // 2], F32, tag="ps")
                    nc.tensor.matmul(
                        rr[:], onerow_sb[:], rs8[0:1, nsl],
                        start=True, stop=True,
                    )
                    nc.vector.tensor_copy(out=rsb[:, nsl], in_=rr[:])
                # scaled-U in fp8: uts = ut * rs8 (per-row) / 8
                for kc in range(2):
                    nc.vector.scalar_tensor_tensor(
                        out=uts[:, kc, :], in0=ut[:, kc, :], scalar=1.0,
                        in1=rsb[:], op0=MULT, op1=MULT,
                    )

            # ---- per row-block: logits+lnC matmul -> exp -> reduce -------
            for col in range(NCOL):
                    i, rb = divmod(col, NBLK)
                    uts = uts_h[i]
                    ph = col % 2
                    if ph == 0:
                        m_sb = m_pool.tile([P, 2, BT], FP8, tag="m")
                        nc.sync.dma_start(out=m_sb[:], in_=mln_d[:, col // 2])
                    lhs = uts[:, :, rb * P:(rb + 1) * P]
                    je = junk.tile([P, BT], BF16, tag="je")
                    for pt_i in range(NPT):
                        ps = psum_s.tile([P, PTILE], F32, tag="ps")
                        for s in range(PTILE // 512):
                            csl = slice(
                                pt_i * PTILE + s * 512, pt_i * PTILE + (s + 1) * 512
                            )
                            psl = slice(s * 512, (s + 1) * 512)
                            nc.tensor.matmul(
                                ps[:, psl], lhs, azt_sb[:, :, csl],
                                start=True, stop=False, perf_mode=DR,
                            )
                            nc.tensor.matmul(
                                ps[:, psl], id8_sb[:, ph], m_sb[:, :, csl],
                                start=False, stop=True, perf_mode=DR,
                            )
                        nc.scalar.activation(
                            out=je[:, pt_i * PTILE:(pt_i + 1) * PTILE], in_=ps[:],
                            func=mybir.ActivationFunctionType.Exp,
                            accum_out=racc_sb[:, col, pt_i:pt_i + 1],
                        )
            # ---- loss = ln(sum_pt R_pt) - pos_logit ----------------------
            nc.vector.tensor_tensor(
                rsum_sb[:], racc_sb[:, :, 0], racc_sb[:, :, 1], ADD
            )
            for pt_i in range(2, NPT):
                nc.vector.tensor_tensor(
                    rsum_sb[:], rsum_sb[:], racc_sb[:, :, pt_i], ADD
                )
            nc.scalar.activation(
                out=loss_sb[:], in_=rsum_sb[:],
                func=mybir.ActivationFunctionType.Ln,
            )
            nc.vector.tensor_tensor(
                loss_sb[:], loss_sb[:], spT_sb[:], mybir.AluOpType.subtract,
            )

        nc.sync.dma_start(out=loss_d[:], in_=loss_sb[:])

    nc.compile()
    _split_multiwait_drains(nc)
    return nc


def prepare_inputs(z_seq, preds, neg_idx):
    """Host-side sharding/packing. Returns (in_maps, valid_counts)."""
    z_flat = np.asarray(z_seq, dtype=np.float32).reshape(BT, D)
    preds = np.asarray(preds, dtype=np.float32)
    neg_idx = np.asarray(neg_idx)

    norms = np.linalg.norm(z_flat, axis=1, keepdims=True)
    az = z_flat / np.maximum(norms, 1e-12)
    azt8 = np.ascontiguousarray(
        (az.T * FP8_SCALE).reshape(2, P, BT).transpose(1, 0, 2)
    ).astype(E4M3)

    # pt[d, i*4+kc*2+mc, e] = preds[i, mc*128+e, kc*128+d]
    pt = np.empty((P, H * 4, P), dtype=ml_dtypes.bfloat16)
    for i in range(H):
        w = preds[i]  # [e_out, d_in]
        for kc in range(2):
            for mc in range(2):
                blk = w[mc * P:(mc + 1) * P, kc * P:(kc + 1) * P]  # [e, d]
                pt[:, i * 4 + kc * 2 + mc, :] = blk.T.astype(ml_dtypes.bfloat16)

    # DoubleRow identity planes for the M-add matmul
    id8 = np.zeros((P, 2, 2, P), dtype=E4M3)
    eye = np.eye(P, dtype=np.float32).astype(E4M3)
    id8[:, 0, 0, :] = eye
    id8[:, 1, 1, :] = eye

    # ln(count) lookup (counts are small ints; avoid a big np.log)
    lut = np.full(260, NEG_M, dtype=np.float32)
    lut[1:] = np.log(np.arange(1, 260, dtype=np.float32))
    lut[0] = NEG_M

    in_maps = []
    valid_counts = np.zeros((N_CORES, H), dtype=np.int64)
    for c in range(N_CORES):
        n0 = c * NROW
        zat = np.zeros((P, H * 2, NROW), dtype=ml_dtypes.bfloat16)
        azp = np.zeros((P, H * 2, NROW), dtype=ml_dtypes.bfloat16)
        mln = np.zeros((P, NPAIR, 2, BT), dtype=E4M3)
        for i, k in enumerate(HORIZONS):
            L = T - k
            BL = B * L
            nvalid = min(max(BL - n0, 0), NROW)
            valid_counts[c, i] = nvalid
            n = n0 + np.arange(NROW)
            nv = n[:nvalid]
            b = nv // L
            a_full = np.zeros(NROW, dtype=np.int64)
            a_full[:nvalid] = nv + b * k          # anchor flat rows
            p_full = np.zeros(NROW, dtype=np.int64)
            p_full[:nvalid] = nv + (b + 1) * k    # positive flat rows
            zat[:, i * 2:(i + 1) * 2, :] = (
                z_flat[a_full].T.reshape(2, P, NROW).transpose(1, 0, 2)
            ).astype(ml_dtypes.bfloat16)
            azp[:, i * 2:(i + 1) * 2, :] = (
                az[p_full].T.reshape(2, P, NROW).transpose(1, 0, 2)
            ).astype(ml_dtypes.bfloat16)

            # integer counts: negatives multiplicity + 1 at the positive
            cm = np.zeros((NROW, BT), dtype=np.int16)
            rows = np.repeat(np.arange(nvalid), N_NEG)
            np.add.at(cm, (rows, neg_idx[i, nv, :].reshape(-1)), 1)
            cm[np.arange(NROW), p_full] += 1
            if nvalid < NROW:
                # pad rows: single count at slot 0 (host ignores them)
                cm[nvalid:] = 0
                cm[nvalid:, 0] = 1
            mm = lut[cm]  # [NROW, BT] f32: ln(count) or NEG_M
            mm8 = mm.astype(E4M3)
            for rb in range(NBLK):
                col = i * NBLK + rb
                mln[:, col // 2, col % 2, :] = mm8[rb * P:(rb + 1) * P]

        in_maps.append(
            {"azt": azt8, "zat": zat, "azp": azp, "pt": pt, "id8": id8, "mln": mln}
        )
    return in_maps, valid_counts


def reduce_outputs(results, valid_counts):
    raw_w = {k: 1.0 / math.sqrt(k) for k in HORIZONS}
    tot_w = sum(raw_w.values())
    total = np.float64(0.0)
    for i, k in enumerate(HORIZONS):
        L = T - k
        BL = B * L
        s = np.float64(0.0)
        for c in range(N_CORES):
            nvalid = int(valid_counts[c, i])
            if nvalid == 0:
                continue
            lm = results[c]["loss"]  # [P, NCOL]
            per_row = lm[:, i * NBLK:(i + 1) * NBLK].T.reshape(NROW)
            s += per_row[:nvalid].sum(dtype=np.float64)
        total += (raw_w[k] / tot_w) * (s / BL)
    return np.float32(total)


_CACHED_NC = None


def kernel(z_seq, preds, neg_idx):
    global _CACHED_NC
    if _CACHED_NC is None:
        _CACHED_NC = build_program()
    nc = _CACHED_NC
    in_maps, valid_counts = prepare_inputs(z_seq, preds, neg_idx)
    res = run_bass_kernel_spmd(nc, in_maps, list(range(N_CORES)))
    return reduce_outputs(res.results, valid_counts)


if __name__ == "__main__":
    rng = np.random.default_rng(0)
    z = rng.standard_normal((B, T, D), dtype=np.float32)
    pr = (rng.standard_normal((H, D, D), dtype=np.float32) / np.sqrt(D)).astype(
        np.float32
    )
    ni = rng.integers(0, BT, size=(H, BT, N_NEG), dtype=np.int64)
    print(kernel(z, pr, ni))
